# revision 16
# baseline (speedup 1.0000x reference)
"""Trainium2 Bass kernel for nn_EternalRecursion (GRUCell self-recursion, B=512, D=500).

Strategy
--------
Data-parallel over 8 NeuronCores: 64 batch rows per core, GRU weights replicated.

Math restructuring (host-side, exact):
  - After step 1 the reference feeds h_new as BOTH x and h of the GRU cell, so
    steps >= 2 use combined weights W_rz = (W_ih+W_hh)[0:1000] for the r/z gates,
    while the n-gate keeps W_ih_n / W_hh_n separate (r multiplies only the h-side).
  - Step 1 (x=state, h=0) uses W_ih with a zero block for the h-side n columns,
    which makes it the *same* device code path with different weights.
  - Biases are folded into the matmul via an extra contraction row of ones.
  - The break check "mean(h_k) > bc" latches the output at the first step k*
    whose global mean exceeds bc. The device free-runs L steps, records per-step
    per-partition sums (free side-output of the last fused DVE op), and the host
    computes the global means. If the break fires before the last step (it cannot
    for the harness inputs), the kernel is re-built with L=k* and re-run.

Device layout (per core, per step) — all matmul operands bf16:
  - h is stored "packed": [128 partitions, 250 free] with partition 64*H+b
    holding h[b, 250*H + c].
  - Two PSUM blocks per step, organized per half-chunk of the packed free dim:
      block A (cols c=0:125):   [r c0 | z c0 | gin c0 | ghn c0]  (N=500)
      block B (cols c=125:250): [r c1 | z c1 | gin c1 | ghn c1]  (N=500)
    so each chunk's full gate chain can start as soon as its block is done.
  - 16 gate matmuls per step: 8 K-groups x 2 blocks, doubled-contraction packing
    (stationary holds h^T twice along K: gate-half G0 channels in array cols
    0:64, G1 in 64:128; groups 0-3 cover D-blocks 0,1 = transpose pA, groups
    4-7 cover D-blocks 2,3 = transpose pB).
  - PE stream order per step (software-pipelined across steps):
      [A u0-3][B u0-3]  (need only pA of prev step)
      [pB-transpose + copies of prev step]
      [A u4-7][B u4-7]  (need pB of prev step)
      [accA: rhnA added into the gin-c0 PSUM region via identity matmul]
      [pA-transpose of this step] ... [accB] ... (pB emitted next iteration)
  - Gate chain per chunk X:  r = sigmoid(gr);  w = sigmoid(-gz) = 1-z;
      wh = w*hprev, zh = hprev - wh         (GPSIMD)
      rhn = r*ghn                           (DVE)
      targ = gin + rhn                      (PE accumulate-matmul into PSUM)
      n = tanh(targ_psum)                   (ACT)
      t2 = n*w                              (DVE)
      hnew = t2 + zh   [+ per-step sums accumulator side-output]   (DVE)
"""

import os
import sys
import types
import numpy as np
import ml_dtypes

NP_BF16 = ml_dtypes.bfloat16

D = 500
B = 512
NCORES = 8
BS = B // NCORES          # 64 batch rows per core
HALF = 250                # free columns of the packed layout
# K permutation: hT column-groups are [0:125 | 250:375 | 125:250 | 375:500]
PERM = np.concatenate([
    np.arange(0, 125), np.arange(250, 375),
    np.arange(125, 250), np.arange(375, 500),
])


def _install_hook_module():
    """Provide antenv.axon_hooks (missing from the RO image) so NTFF tracing
    through bass_utils can work when requested. Harmless if anything fails."""
    if "antenv.axon_hooks" in sys.modules:
        return
    mod = types.ModuleType("antenv.axon_hooks")
    holder = [None]
    mod.set_axon_ntff_profile_hook = lambda h: holder.__setitem__(0, h)
    mod.get_axon_ntff_profile_hook = lambda: holder[0]
    sys.modules["antenv.axon_hooks"] = mod
    try:
        from trn_agent_boot.trn_boot import _ntff_profile_via_ctypes
        hook = _ntff_profile_via_ctypes("/opt/axon/libaxon_pjrt.so")
        mod.set_axon_ntff_profile_hook(hook)
    except Exception:
        pass


_install_hook_module()

import concourse.bass as bass  # noqa: E402
import concourse.mybir as mybir  # noqa: E402
import concourse.tile as tile  # noqa: E402
from concourse import bass_utils  # noqa: E402
from concourse.masks import make_identity  # noqa: E402
import bass_rust  # noqa: E402

F32 = mybir.dt.float32
BF16 = mybir.dt.bfloat16
AF = mybir.ActivationFunctionType
ALU = mybir.AluOpType


def _split_overwide_waits(nc, maxw=1):
    """walrus here rejects >1 sync wait per instruction; spread extras over
    preceding NoOp carriers. Most multi-wait instructions get same-engine
    carriers (order-preserving); the kernel-end drain (many loose-end waits)
    gets carriers round-robined across all engines so they resolve in
    parallel before the final barrier instead of serially on one engine."""
    n_new = 0
    all_engines = (mybir.EngineType.SP, mybir.EngineType.Activation,
                   mybir.EngineType.PE, mybir.EngineType.DVE,
                   mybir.EngineType.Pool)
    for fn in nc.m.functions:
        for bb in fn.blocks:
            out = []
            for inst in bb.instructions:
                si = inst.sync_info
                if si is not None and si.on_wait and len(si.on_wait) > maxw:
                    waits = list(si.on_wait)
                    chunks = [waits[i:i + maxw] for i in range(0, len(waits), maxw)]
                    spread = len(chunks) > 4  # only the big end-of-kernel drain
                    for j, ch in enumerate(chunks[:-1]):
                        eng = all_engines[j % len(all_engines)] if spread \
                            else inst.engine
                        nd = mybir.InstNoOp(
                            name=f"I-swx{n_new}", engine=eng,
                            bass_nofuse=True,
                            sync_info=bass_rust.SyncInfo(on_wait=ch, on_update=[]))
                        n_new += 1
                        nc.register_instruction(nd, overwrite=True)
                        out.append(nd)
                    inst.sync_info = bass_rust.SyncInfo(
                        on_wait=chunks[-1], on_update=list(si.on_update or []))
                out.append(inst)
            bb.instructions = out
    return n_new


def _build(L):
    """Build the Bass module for L GRU steps. Returns nc."""
    assert L >= 1
    nc = bass.Bass("TRN2", target_bir_lowering=False, debug=False)

    statet_d = nc.dram_tensor("statet", [126, 1024], BF16, kind="ExternalInput").ap()
    wa_d = nc.dram_tensor("wa", [2, 126, 4000], BF16, kind="ExternalInput").ap()
    wb_d = nc.dram_tensor("wb", [2, 126, 4000], BF16, kind="ExternalInput").ap()
    hout_d = nc.dram_tensor("hout", [128, HALF], BF16, kind="ExternalOutput").ap()
    sums_d = nc.dram_tensor("sums", [128, 2 * L], F32, kind="ExternalOutput").ap()

    with tile.TileContext(nc) as tc:
        import contextlib
        with contextlib.ExitStack() as ctx:
            consts = ctx.enter_context(tc.tile_pool(name="consts", bufs=1))
            wpool = ctx.enter_context(tc.tile_pool(name="weights", bufs=1))
            hpool = ctx.enter_context(tc.tile_pool(name="hstate", bufs=1))
            work = ctx.enter_context(tc.tile_pool(name="work", bufs=2))
            gpsum = ctx.enter_context(tc.tile_pool(name="gpsum", bufs=2, space="PSUM"))
            tpsum = ctx.enter_context(tc.tile_pool(name="tpsum", bufs=2, space="PSUM"))

            identity = consts.tile([128, 128], F32, tag="identity", name="identity")
            make_identity(nc, identity[:])
            identity_b = consts.tile([128, 128], BF16, tag="identity_b", name="identity_b")
            nc.vector.tensor_copy(identity_b[:], identity[:])

            statet = wpool.tile([126, 1024], BF16, tag="statet", name="statet")
            nc.gpsimd.dma_start(statet[:], statet_d)
            # fused weight loads: 4 large DMAs
            wa_t = [wpool.tile([126, 4000], BF16, tag=f"wah{h}", name=f"wah{h}")
                    for h in range(2)]
            wb_t = [wpool.tile([126, 4000], BF16, tag=f"wbh{h}", name=f"wbh{h}")
                    for h in range(2)]
            nc.gpsimd.dma_start(wa_t[0][:], wa_d[0])
            nc.sync.dma_start(wa_t[1][:], wa_d[1])
            nc.gpsimd.dma_start(wb_t[0][:], wb_d[0])
            nc.sync.dma_start(wb_t[1][:], wb_d[1])
            wa = [wa_t[u // 4][:, 1000 * (u % 4):1000 * (u % 4 + 1)] for u in range(8)]
            wb = [wb_t[u // 4][:, 1000 * (u % 4):1000 * (u % 4 + 1)] for u in range(8)]

            hT = [hpool.tile([126, 1024], BF16, tag="hta", name="hta"),
                  hpool.tile([126, 1024], BF16, tag="htb", name="htb")]
            # zero-init (the complementary halves of each K-group must stay 0),
            # then DMA row 125 from the statet image (ones in groups 1 and 3;
            # DVE ops can't start at partition 125, DMA can).
            nc.vector.memzero(hT[0][0:125, :])
            nc.vector.memzero(hT[1][0:125, :])
            nc.gpsimd.dma_start(hT[0][125:126, :], statet_d[125:126, :])
            nc.gpsimd.dma_start(hT[1][125:126, :], statet_d[125:126, :])

            sums = consts.tile([128, 2 * L], F32, tag="sums", name="sums")

            def ku_of(u):
                return 126 if u in (1, 3) else 125

            hprev = None
            hnew = None
            pend_pB = None   # closure emitting prev step's pB transpose+copies
            for k in range(1, L + 1):
                first = k == 1
                lhs_tile = statet if first else hT[k % 2]
                W = wa if first else wb

                gA = gpsum.tile([128, 512], F32, tag="gA", name="gA")
                gB = gpsum.tile([128, 512], F32, tag="gB", name="gB")

                def mms(out_ap, c0, us, start, stop):
                    for i, u in enumerate(us):
                        ku = ku_of(u)
                        nc.tensor.matmul(out_ap, lhs_tile[0:ku, 128 * u:128 * u + 128],
                                         W[u][0:ku, c0:c0 + 500],
                                         start=start and i == 0,
                                         stop=stop and i == len(us) - 1,
                                         skip_group_check=True)

                # phase 1: groups 0-3 (depend only on prev pA copies)
                mms(gA[:, 0:500], 0, range(4), True, False)
                mms(gB[:, 0:500], 500, range(4), True, False)
                # prev step's pB transpose + copies (needs prev chain B only)
                if pend_pB is not None:
                    pend_pB()
                    pend_pB = None
                # phase 2: groups 4-7 (depend on prev pB copies)
                mms(gA[:, 0:500], 0, range(4, 8), False, True)
                mms(gB[:, 0:500], 500, range(4, 8), False, True)

                # ---- chains ----
                r = work.tile([128, HALF], BF16, tag="r", name="r")
                w = work.tile([128, HALF], BF16, tag="w", name="w")
                wh = work.tile([128, HALF], BF16, tag="wh", name="wh")
                zh = work.tile([128, HALF], BF16, tag="zh", name="zh")
                rhn = work.tile([128, HALF], BF16, tag="rhn", name="rhn")
                n = work.tile([128, HALF], BF16, tag="n", name="n")
                t2 = work.tile([128, HALF], BF16, tag="t2", name="t2")
                hnew = work.tile([128, HALF], BF16, tag="hnew", name="hnew")
                if k < L:
                    dst = hT[(k + 1) % 2]
                    dstv = dst[:].rearrange("p (u c) -> p u c", c=128)

                def chain(cx, g):
                    s = slice(125 * cx, 125 * (cx + 1))
                    acc = sums[:, 2 * (k - 1) + cx:2 * (k - 1) + cx + 1]
                    nc.scalar.activation(r[:, s], g[:, 0:125], AF.Sigmoid)
                    nc.scalar.activation(w[:, s], g[:, 125:250], AF.Sigmoid,
                                         scale=-1.0)
                    if not first:
                        nc.gpsimd.tensor_mul(wh[:, s], w[:, s], hprev[:, s])
                        nc.gpsimd.tensor_sub(zh[:, s], hprev[:, s], wh[:, s])
                    nc.vector.tensor_mul(rhn[:, s], r[:, s], g[:, 375:500])
                    # targ = gin + rhn via PE accumulation into the gin region
                    nc.tensor.matmul(g[:, 250:375], identity_b[:], rhn[:, s],
                                     start=False, stop=True,
                                     skip_group_check=True)
                    nc.scalar.activation(n[:, s], g[:, 250:375], AF.Tanh)
                    nc.vector.tensor_mul(t2[:, s], n[:, s], w[:, s])
                    if first:
                        nc.vector.scalar_tensor_tensor(
                            hnew[:, s], t2[:, s], 0.0, t2[:, s],
                            op0=ALU.mult, op1=ALU.add, accum_out=acc)
                    else:
                        nc.vector.scalar_tensor_tensor(
                            hnew[:, s], t2[:, s], 0.0, zh[:, s],
                            op0=ALU.bypass, op1=ALU.add, accum_out=acc)

                def trans(cx, tag):
                    # pX = transpose of hnew chunk cx, then copies into hT
                    pX = tpsum.tile([125, 128], F32, tag="pT", name=tag)
                    nc.tensor.matmul(pX[:], hnew[:, 125 * cx:125 * (cx + 1)],
                                     identity_b[:])
                    pXv = pX[:].rearrange("p (u c) -> p u c", c=64)
                    ublk = 4 * cx
                    nc.scalar.copy(dstv[0:125, ublk:ublk + 2, 0:64], pXv)
                    nc.vector.tensor_copy(dstv[0:125, ublk + 2:ublk + 4, 64:128], pXv)

                chain(0, gA)
                if k < L:
                    trans(0, "pA")
                chain(1, gB)
                if k < L:
                    # defer pB transpose+copies into next step's MM stream
                    def mk(hn, dv, kk):
                        def emit():
                            pX = tpsum.tile([125, 128], F32, tag="pT", name=f"pB{kk}")
                            nc.tensor.matmul(pX[:], hn[:, 125:250], identity_b[:])
                            pXv = pX[:].rearrange("p (u c) -> p u c", c=64)
                            nc.scalar.copy(dv[0:125, 4:6, 0:64], pXv)
                            nc.vector.tensor_copy(dv[0:125, 6:8, 64:128], pXv)
                        return emit
                    pend_pB = mk(hnew, dstv, k)
                hprev = hnew

            if pend_pB is not None:   # pragma: no cover (k==L skips it)
                pend_pB()

            nc.gpsimd.dma_start(hout_d, hnew[:])
            nc.gpsimd.dma_start(sums_d, sums[:])

    _split_overwide_waits(nc)
    return nc


_NC_CACHE = {}


def _get_nc(L):
    if L not in _NC_CACHE:
        _NC_CACHE[L] = _build(L)
    return _NC_CACHE[L]


def _prep_weights(W_ih, W_hh, b_ih, b_hh):
    """Build wa/wb DRAM images [2, 126, 4000] (grouped, permuted, bias rows).

    Per group u = (D-block t, gate-half G), the [126, 1000] W image columns:
      [0:500]    block A (chunk c0): [Wr[250G+0:125] | Wz | Win | Whn]
      [500:1000] block B (chunk c1): same with row offset +125
    rows 0:125 = channels PERM-block t; row 125 = bias (groups 1 and 3 only).
    """
    W_ih = np.asarray(W_ih, np.float32)
    W_hh = np.asarray(W_hh, np.float32)
    b_ih = np.asarray(b_ih, np.float32)
    b_hh = np.asarray(b_hh, np.float32)

    DBLK = (0, 1, 0, 1, 2, 3, 2, 3)
    GHALF = (0, 0, 1, 1, 0, 0, 1, 1)

    def pack(Wr, Wz, Win, Whn, br, bz, bin_, bhn):
        out = np.zeros((8, 126, 1000), np.float32)
        for u in range(8):
            t, G = DBLK[u], GHALF[u]
            ch = PERM[125 * t:125 * (t + 1)]          # channel ids of this block
            for blk, off in ((0, 0), (1, 125)):
                base = 500 * blk
                rows = slice(250 * G + off, 250 * G + off + 125)
                out[u, 0:125, base + 0:base + 125] = Wr[rows][:, ch].T
                out[u, 0:125, base + 125:base + 250] = Wz[rows][:, ch].T
                out[u, 0:125, base + 250:base + 375] = Win[rows][:, ch].T
                out[u, 0:125, base + 375:base + 500] = Whn[rows][:, ch].T
        for u in (1, 3):
            G = GHALF[u]
            for blk, off in ((0, 0), (1, 125)):
                base = 500 * blk
                rows = slice(250 * G + off, 250 * G + off + 125)
                out[u, 125, base + 0:base + 125] = br[rows]
                out[u, 125, base + 125:base + 250] = bz[rows]
                out[u, 125, base + 250:base + 375] = bin_[rows]
                out[u, 125, base + 375:base + 500] = bhn[rows]
        return np.ascontiguousarray(
            out.reshape(2, 4, 126, 1000).transpose(0, 2, 1, 3).reshape(2, 126, 4000)
            .astype(NP_BF16))

    zeros = np.zeros((500, D), np.float32)
    zb = np.zeros(500, np.float32)
    br = b_ih[0:500] + b_hh[0:500]
    bz = b_ih[500:1000] + b_hh[500:1000]
    bin_ = b_ih[1000:1500]
    bhn = b_hh[1000:1500]

    WB = pack(W_ih[0:500] + W_hh[0:500], W_ih[500:1000] + W_hh[500:1000],
              W_ih[1000:1500], W_hh[1000:1500], br, bz, bin_, bhn)
    WA = pack(W_ih[0:500], W_ih[500:1000],
              W_ih[1000:1500], zeros, br, bz, bin_, bhn)
    return WA, WB


def _prep_state(state):
    """Per-core stationary state^T images [126, 1024]."""
    state = np.asarray(state, np.float32)
    outs = []
    DBLK = (0, 1, 0, 1, 2, 3, 2, 3)
    GHALF = (0, 0, 1, 1, 0, 0, 1, 1)
    for c in range(NCORES):
        shard = state[BS * c:BS * (c + 1)]            # [64, 500]
        st = shard[:, PERM].T                         # [500, 64]
        img = np.zeros((126, 1024), np.float32)
        for u in range(8):
            rows = st[125 * DBLK[u]:125 * (DBLK[u] + 1)]
            off = 128 * u + 64 * GHALF[u]
            img[0:125, off:off + 64] = rows
        img[125, 128 * 1:128 * 1 + 64] = 1.0
        img[125, 128 * 3 + 64:128 * 3 + 128] = 1.0
        outs.append(img.astype(NP_BF16))
    return outs


def _run(L, stateTs, wa, wb, trace=False):
    nc = _get_nc(L)
    in_maps = [{"statet": np.ascontiguousarray(stateTs[c]),
                "wa": wa, "wb": wb} for c in range(NCORES)]
    res = bass_utils.run_bass_kernel_spmd(
        nc, in_maps, core_ids=list(range(NCORES)), trace=trace)
    shards = []
    sums = np.zeros((128, 2 * L), np.float64)
    for c in range(NCORES):
        hout = res.results[c]["hout"].astype(np.float32)
        shards.append(np.concatenate([hout[0:64], hout[64:128]], axis=1))
        sums += res.results[c]["sums"].astype(np.float64)
    h = np.concatenate(shards, axis=0)                # [512, 500]
    means = (sums[:, 0::2] + sums[:, 1::2]).sum(axis=0) / (B * D)  # [L]
    return h, means, res


def kernel(state, W_ih, W_hh, b_ih, b_hh, break_condition, recursion_limit):
    state = np.asarray(state, np.float32)
    L = int(np.asarray(recursion_limit))
    if L <= 0:
        return state.copy()
    bc = float(np.asarray(break_condition))

    wa, wb = _prep_weights(W_ih, W_hh, b_ih, b_hh)
    stateTs = _prep_state(state)

    h, means, _ = _run(L, stateTs, wa, wb)
    fired = np.nonzero(means > bc)[0]
    if fired.size and fired[0] + 1 < L:
        # break fired at step k* = fired[0]+1: output latches h_{k*}
        h, _, _ = _run(int(fired[0]) + 1, stateTs, wa, wb)
    return h.astype(np.float32)


# revision 19
# speedup vs baseline: 1.2020x; 1.2020x over previous
"""Trainium2 Bass kernel for nn_EternalRecursion (GRUCell self-recursion, B=512, D=500).

Strategy
--------
Data-parallel over 8 NeuronCores: 64 batch rows per core, GRU weights replicated.

Math restructuring (host-side, exact):
  - After step 1 the reference feeds h_new as BOTH x and h of the GRU cell, so
    steps >= 2 use combined weights W_rz = (W_ih+W_hh)[0:1000] for the r/z gates,
    while the n-gate keeps W_ih_n / W_hh_n separate (r multiplies only the h-side).
  - Step 1 (x=state, h=0) uses W_ih with a zero block for the h-side n columns,
    which makes it the *same* device code path with different weights.
  - Biases are folded into the matmul via an extra contraction row of ones.
  - The break check "mean(h_k) > bc" latches the output at the first step k*
    whose global mean exceeds bc. The device free-runs L steps, records per-step
    per-partition sums (free side-output of the last fused DVE op), and the host
    computes the global means. If the break fires before the last step (it cannot
    for the harness inputs), the kernel is re-built with L=k* and re-run.

Device layout (per core, per step) — all matmul operands bf16:
  - h is stored "packed": [128 partitions, 250 free] with partition 64*H+b
    holding h[b, 250*H + c].
  - Two PSUM blocks per step, organized per half-chunk of the packed free dim:
      block A (cols c=0:125):   [r c0 | z c0 | gin c0 | ghn c0]  (N=500)
      block B (cols c=125:250): [r c1 | z c1 | gin c1 | ghn c1]  (N=500)
    so each chunk's full gate chain can start as soon as its block is done.
  - 16 gate matmuls per step: 8 K-groups x 2 blocks, doubled-contraction packing
    (stationary holds h^T twice along K: gate-half G0 channels in array cols
    0:64, G1 in 64:128; groups 0-3 cover D-blocks 0,1 = transpose pA, groups
    4-7 cover D-blocks 2,3 = transpose pB).
  - PE stream order per step (software-pipelined across steps):
      [A u0-3][B u0-3]  (need only pA of prev step)
      [pB-transpose + copies of prev step]
      [A u4-7][B u4-7]  (need pB of prev step)
      [accA: rhnA added into the gin-c0 PSUM region via identity matmul]
      [pA-transpose of this step] ... [accB] ... (pB emitted next iteration)
  - Gate chain per chunk X:  r = sigmoid(gr);  w = sigmoid(-gz) = 1-z;
      wh = w*hprev, zh = hprev - wh         (GPSIMD)
      rhn = r*ghn                           (DVE)
      targ = gin + rhn                      (PE accumulate-matmul into PSUM)
      n = tanh(targ_psum)                   (ACT)
      t2 = n*w                              (DVE)
      hnew = t2 + zh   [+ per-step sums accumulator side-output]   (DVE)
"""

import os
import sys
import types
import numpy as np
import ml_dtypes

NP_BF16 = ml_dtypes.bfloat16

D = 500
B = 512
NCORES = 8
BS = B // NCORES          # 64 batch rows per core
HALF = 250                # free columns of the packed layout
# K permutation: hT column-groups are [0:125 | 250:375 | 125:250 | 375:500]
PERM = np.concatenate([
    np.arange(0, 125), np.arange(250, 375),
    np.arange(125, 250), np.arange(375, 500),
])


def _install_hook_module():
    """Provide antenv.axon_hooks (missing from the RO image) so NTFF tracing
    through bass_utils can work when requested. Harmless if anything fails."""
    if "antenv.axon_hooks" in sys.modules:
        return
    mod = types.ModuleType("antenv.axon_hooks")
    holder = [None]
    mod.set_axon_ntff_profile_hook = lambda h: holder.__setitem__(0, h)
    mod.get_axon_ntff_profile_hook = lambda: holder[0]
    sys.modules["antenv.axon_hooks"] = mod
    try:
        from trn_agent_boot.trn_boot import _ntff_profile_via_ctypes
        hook = _ntff_profile_via_ctypes("/opt/axon/libaxon_pjrt.so")
        mod.set_axon_ntff_profile_hook(hook)
    except Exception:
        pass


_install_hook_module()

import concourse.bass as bass  # noqa: E402
import concourse.mybir as mybir  # noqa: E402
import concourse.tile as tile  # noqa: E402
from concourse import bass_utils  # noqa: E402
from concourse.masks import make_identity  # noqa: E402
import bass_rust  # noqa: E402

F32 = mybir.dt.float32
BF16 = mybir.dt.bfloat16
AF = mybir.ActivationFunctionType
ALU = mybir.AluOpType


def _split_overwide_waits(nc, maxw=1):
    """walrus here rejects >1 sync wait per instruction; spread extras over
    preceding NoOp carriers. Most multi-wait instructions get same-engine
    carriers (order-preserving); the kernel-end drain (many loose-end waits)
    gets carriers round-robined across all engines so they resolve in
    parallel before the final barrier instead of serially on one engine."""
    n_new = 0
    all_engines = (mybir.EngineType.SP, mybir.EngineType.Activation,
                   mybir.EngineType.PE, mybir.EngineType.DVE,
                   mybir.EngineType.Pool)
    for fn in nc.m.functions:
        for bb in fn.blocks:
            out = []
            for inst in bb.instructions:
                si = inst.sync_info
                if si is not None and si.on_wait and len(si.on_wait) > maxw:
                    waits = list(si.on_wait)
                    chunks = [waits[i:i + maxw] for i in range(0, len(waits), maxw)]
                    spread = len(chunks) > 4  # only the big end-of-kernel drain
                    for j, ch in enumerate(chunks[:-1]):
                        eng = all_engines[j % len(all_engines)] if spread \
                            else inst.engine
                        nd = mybir.InstNoOp(
                            name=f"I-swx{n_new}", engine=eng,
                            bass_nofuse=True,
                            sync_info=bass_rust.SyncInfo(on_wait=ch, on_update=[]))
                        n_new += 1
                        nc.register_instruction(nd, overwrite=True)
                        out.append(nd)
                    inst.sync_info = bass_rust.SyncInfo(
                        on_wait=chunks[-1], on_update=list(si.on_update or []))
                out.append(inst)
            bb.instructions = out
    return n_new


def _build(L):
    """Build the Bass module for L GRU steps. Returns nc."""
    assert L >= 1
    nc = bass.Bass("TRN2", target_bir_lowering=False, debug=False)

    statet_d = nc.dram_tensor("statet", [126, 1024], BF16, kind="ExternalInput").ap()
    wa_d = nc.dram_tensor("wa", [2, 126, 4000], BF16, kind="ExternalInput").ap()
    wb_d = nc.dram_tensor("wb", [2, 126, 4000], BF16, kind="ExternalInput").ap()
    hout_d = nc.dram_tensor("hout", [128, HALF], BF16, kind="ExternalOutput").ap()
    sums_d = nc.dram_tensor("sums", [128, 2 * L], F32, kind="ExternalOutput").ap()

    with tile.TileContext(nc) as tc:
        import contextlib
        with contextlib.ExitStack() as ctx:
            consts = ctx.enter_context(tc.tile_pool(name="consts", bufs=1))
            wpool = ctx.enter_context(tc.tile_pool(name="weights", bufs=1))
            hpool = ctx.enter_context(tc.tile_pool(name="hstate", bufs=1))
            work = ctx.enter_context(tc.tile_pool(name="work", bufs=2))
            gpsum = ctx.enter_context(tc.tile_pool(name="gpsum", bufs=2, space="PSUM"))
            tpsum = ctx.enter_context(tc.tile_pool(name="tpsum", bufs=2, space="PSUM"))
            dpsum = ctx.enter_context(tc.tile_pool(name="dpsum", bufs=1, space="PSUM"))

            identity = consts.tile([128, 128], F32, tag="identity", name="identity")
            make_identity(nc, identity[:])
            identity_b = consts.tile([128, 128], BF16, tag="identity_b", name="identity_b")
            nc.vector.tensor_copy(identity_b[:], identity[:])
            # negated identity: lets a matmul SUBTRACT a transpose (pX -= wh^T)
            identity_n = consts.tile([128, 128], BF16, tag="identity_n", name="identity_n")
            nc.vector.tensor_scalar_mul(identity_n[:], identity[:], -1.0)

            statet = wpool.tile([126, 1024], BF16, tag="statet", name="statet")
            nc.gpsimd.dma_start(statet[:], statet_d)
            # fused weight loads: 4 large DMAs
            wa_t = [wpool.tile([126, 4000], BF16, tag=f"wah{h}", name=f"wah{h}")
                    for h in range(2)]
            wb_t = [wpool.tile([126, 4000], BF16, tag=f"wbh{h}", name=f"wbh{h}")
                    for h in range(2)]
            nc.gpsimd.dma_start(wa_t[0][:], wa_d[0])
            nc.sync.dma_start(wa_t[1][:], wa_d[1])
            nc.gpsimd.dma_start(wb_t[0][:], wb_d[0])
            nc.sync.dma_start(wb_t[1][:], wb_d[1])
            wa = [wa_t[u // 4][:, 1000 * (u % 4):1000 * (u % 4 + 1)] for u in range(8)]
            wb = [wb_t[u // 4][:, 1000 * (u % 4):1000 * (u % 4 + 1)] for u in range(8)]

            hT = [hpool.tile([126, 1024], BF16, tag="hta", name="hta"),
                  hpool.tile([126, 1024], BF16, tag="htb", name="htb")]
            # zero-init (the complementary halves of each K-group must stay 0),
            # then DMA row 125 from the statet image (ones in groups 1 and 3;
            # DVE ops can't start at partition 125, DMA can).
            nc.vector.memzero(hT[0][0:125, :])
            nc.vector.memzero(hT[1][0:125, :])
            nc.gpsimd.dma_start(hT[0][125:126, :], statet_d[125:126, :])
            nc.gpsimd.dma_start(hT[1][125:126, :], statet_d[125:126, :])

            sums = consts.tile([128, 2 * L], F32, tag="sums", name="sums")

            def ku_of(u):
                return 126 if u in (1, 3) else 125

            hprev = None
            hnew = None
            pend_pB = None   # closure emitting prev step's pB transpose+copies
            for k in range(1, L + 1):
                first = k == 1
                lhs_tile = statet if first else hT[k % 2]
                W = wa if first else wb

                gA = gpsum.tile([128, 512], F32, tag="gA", name="gA")
                gB = gpsum.tile([128, 512], F32, tag="gB", name="gB")

                def mms(out_ap, c0, us, start, stop):
                    for i, u in enumerate(us):
                        ku = ku_of(u)
                        nc.tensor.matmul(out_ap, lhs_tile[0:ku, 128 * u:128 * u + 128],
                                         W[u][0:ku, c0:c0 + 500],
                                         start=start and i == 0,
                                         stop=stop and i == len(us) - 1,
                                         skip_group_check=True)

                # phase 1: groups 0-3 (depend only on prev pA copies)
                mms(gA[:, 0:500], 0, range(4), True, False)
                mms(gB[:, 0:500], 500, range(4), True, False)
                # prev step's pB transpose + copies (needs prev chain B only)
                if pend_pB is not None:
                    pend_pB()
                    pend_pB = None
                # phase 2: groups 4-7 (depend on prev pB copies)
                mms(gA[:, 0:500], 0, range(4, 8), False, True)
                mms(gB[:, 0:500], 500, range(4, 8), False, True)

                # ---- chains ----
                r = work.tile([128, HALF], BF16, tag="r", name="r")
                w = work.tile([128, HALF], BF16, tag="w", name="w")
                wh = work.tile([128, HALF], BF16, tag="wh", name="wh")
                rhn = work.tile([128, HALF], BF16, tag="rhn", name="rhn")
                n = work.tile([128, HALF], BF16, tag="n", name="n")
                t2 = work.tile([128, HALF], BF16, tag="t2", name="t2")
                v = work.tile([128, HALF], BF16, tag="v", name="v")
                hnew = work.tile([128, HALF], BF16, tag="hnew", name="hnew")
                if k < L:
                    dst = hT[(k + 1) % 2]
                    dstv = dst[:].rearrange("p (u c) -> p u c", c=128)

                def dummy(anchor, s):
                    # write-only matmul on a mid-chain tensor: real PE activity
                    # spaced through the serial tail so HAM never re-throttles
                    dmy = dpsum.tile([1, 512], F32, tag="dmy", name="dmy")
                    nc.tensor.matmul(dmy[:], anchor[0:126, s.start:s.start + 1],
                                     wb_t[0][0:126, 0:512])

                def chain(cx, g):
                    s = slice(125 * cx, 125 * (cx + 1))
                    nc.scalar.activation(r[:, s], g[:, 0:125], AF.Sigmoid)
                    # w = sigmoid(-gz) = 1 - z
                    nc.scalar.activation(w[:, s], g[:, 125:250], AF.Sigmoid,
                                         scale=-1.0)
                    if not first:
                        nc.gpsimd.tensor_mul(wh[:, s], w[:, s], hprev[:, s])
                    nc.vector.tensor_mul(rhn[:, s], r[:, s], g[:, 375:500])
                    # targ = gin + rhn via PE accumulation into the gin region
                    nc.tensor.matmul(g[:, 250:375], identity_b[:], rhn[:, s],
                                     start=False, stop=True,
                                     skip_group_check=True)
                    nc.scalar.activation(n[:, s], g[:, 250:375], AF.Tanh)
                    dummy(n, s)
                    nc.vector.tensor_mul(t2[:, s], n[:, s], w[:, s])
                    # lazy packed hnew = t2 - wh + hprev (off the critical path;
                    # only needed by next step's wh, the sums, and the output)
                    acc = sums[:, 2 * (k - 1) + cx:2 * (k - 1) + cx + 1]
                    if first:
                        nc.vector.scalar_tensor_tensor(
                            hnew[:, s], t2[:, s], 0.0, t2[:, s],
                            op0=ALU.mult, op1=ALU.add, accum_out=acc)
                    else:
                        nc.vector.tensor_sub(v[:, s], t2[:, s], wh[:, s])
                        nc.vector.scalar_tensor_tensor(
                            hnew[:, s], v[:, s], 0.0, hprev[:, s],
                            op0=ALU.bypass, op1=ALU.add, accum_out=acc)

                def trans(cx, tag):
                    # pX = hnew^T chunk cx built by accumulating matmuls:
                    #   pX = t2^T - wh^T + hprev^T, with hprev^T taken from the
                    # current stationary image (it IS last step's transpose).
                    s = slice(125 * cx, 125 * (cx + 1))
                    pX = tpsum.tile([125, 128], F32, tag="pT", name=tag)
                    nc.tensor.matmul(pX[:], t2[:, s], identity_b[:],
                                     start=True, stop=first,
                                     skip_group_check=True)
                    if not first:
                        nc.tensor.matmul(pX[:], wh[:, s], identity_n[:],
                                         start=False, stop=False,
                                         skip_group_check=True)
                        g0, g1 = (0, 1) if cx == 0 else (4, 5)
                        nc.tensor.matmul(pX[:, 0:64], identity_b[0:125, 0:125],
                                         lhs_tile[0:125, 128 * g0:128 * g0 + 64],
                                         start=False, stop=False,
                                         skip_group_check=True)
                        nc.tensor.matmul(pX[:, 64:128], identity_b[0:125, 0:125],
                                         lhs_tile[0:125, 128 * g1:128 * g1 + 64],
                                         start=False, stop=True,
                                         skip_group_check=True)
                    pXv = pX[:].rearrange("p (u c) -> p u c", c=64)
                    ublk = 4 * cx
                    nc.scalar.copy(dstv[0:125, ublk:ublk + 2, 0:64], pXv)
                    nc.vector.tensor_copy(dstv[0:125, ublk + 2:ublk + 4, 64:128], pXv)

                chain(0, gA)
                if k < L:
                    trans(0, "pA")
                chain(1, gB)
                if k < L:
                    # defer pB transpose+copies into next step's MM stream
                    def mk(t2_, wh_, dv, lhs, fst, kk):
                        def emit():
                            pX = tpsum.tile([125, 128], F32, tag="pT", name=f"pB{kk}")
                            nc.tensor.matmul(pX[:], t2_[:, 125:250], identity_b[:],
                                             start=True, stop=fst,
                                             skip_group_check=True)
                            if not fst:
                                nc.tensor.matmul(pX[:], wh_[:, 125:250], identity_n[:],
                                                 start=False, stop=False,
                                                 skip_group_check=True)
                                nc.tensor.matmul(pX[:, 0:64], identity_b[0:125, 0:125],
                                                 lhs[0:125, 128 * 4:128 * 4 + 64],
                                                 start=False, stop=False,
                                                 skip_group_check=True)
                                nc.tensor.matmul(pX[:, 64:128], identity_b[0:125, 0:125],
                                                 lhs[0:125, 128 * 5:128 * 5 + 64],
                                                 start=False, stop=True,
                                                 skip_group_check=True)
                            pXv = pX[:].rearrange("p (u c) -> p u c", c=64)
                            nc.scalar.copy(dv[0:125, 4:6, 0:64], pXv)
                            nc.vector.tensor_copy(dv[0:125, 6:8, 64:128], pXv)
                        return emit
                    pend_pB = mk(t2, wh, dstv, lhs_tile, first, k)
                hprev = hnew

            if pend_pB is not None:   # pragma: no cover (k==L skips it)
                pend_pB()

            nc.gpsimd.dma_start(hout_d, hnew[:])
            nc.gpsimd.dma_start(sums_d, sums[:])

    _split_overwide_waits(nc)
    return nc


_NC_CACHE = {}


def _get_nc(L):
    if L not in _NC_CACHE:
        _NC_CACHE[L] = _build(L)
    return _NC_CACHE[L]


def _prep_weights(W_ih, W_hh, b_ih, b_hh):
    """Build wa/wb DRAM images [2, 126, 4000] (grouped, permuted, bias rows).

    Per group u = (D-block t, gate-half G), the [126, 1000] W image columns:
      [0:500]    block A (chunk c0): [Wr[250G+0:125] | Wz | Win | Whn]
      [500:1000] block B (chunk c1): same with row offset +125
    rows 0:125 = channels PERM-block t; row 125 = bias (groups 1 and 3 only).
    """
    W_ih = np.asarray(W_ih, np.float32)
    W_hh = np.asarray(W_hh, np.float32)
    b_ih = np.asarray(b_ih, np.float32)
    b_hh = np.asarray(b_hh, np.float32)

    DBLK = (0, 1, 0, 1, 2, 3, 2, 3)
    GHALF = (0, 0, 1, 1, 0, 0, 1, 1)

    def pack(Wr, Wz, Win, Whn, br, bz, bin_, bhn):
        out = np.zeros((8, 126, 1000), np.float32)
        for u in range(8):
            t, G = DBLK[u], GHALF[u]
            ch = PERM[125 * t:125 * (t + 1)]          # channel ids of this block
            for blk, off in ((0, 0), (1, 125)):
                base = 500 * blk
                rows = slice(250 * G + off, 250 * G + off + 125)
                out[u, 0:125, base + 0:base + 125] = Wr[rows][:, ch].T
                out[u, 0:125, base + 125:base + 250] = Wz[rows][:, ch].T
                out[u, 0:125, base + 250:base + 375] = Win[rows][:, ch].T
                out[u, 0:125, base + 375:base + 500] = Whn[rows][:, ch].T
        for u in (1, 3):
            G = GHALF[u]
            for blk, off in ((0, 0), (1, 125)):
                base = 500 * blk
                rows = slice(250 * G + off, 250 * G + off + 125)
                out[u, 125, base + 0:base + 125] = br[rows]
                out[u, 125, base + 125:base + 250] = bz[rows]
                out[u, 125, base + 250:base + 375] = bin_[rows]
                out[u, 125, base + 375:base + 500] = bhn[rows]
        return np.ascontiguousarray(
            out.reshape(2, 4, 126, 1000).transpose(0, 2, 1, 3).reshape(2, 126, 4000)
            .astype(NP_BF16))

    zeros = np.zeros((500, D), np.float32)
    zb = np.zeros(500, np.float32)
    br = b_ih[0:500] + b_hh[0:500]
    bz = b_ih[500:1000] + b_hh[500:1000]
    bin_ = b_ih[1000:1500]
    bhn = b_hh[1000:1500]

    WB = pack(W_ih[0:500] + W_hh[0:500], W_ih[500:1000] + W_hh[500:1000],
              W_ih[1000:1500], W_hh[1000:1500], br, bz, bin_, bhn)
    WA = pack(W_ih[0:500], W_ih[500:1000],
              W_ih[1000:1500], zeros, br, bz, bin_, bhn)
    return WA, WB


def _prep_state(state):
    """Per-core stationary state^T images [126, 1024]."""
    state = np.asarray(state, np.float32)
    outs = []
    DBLK = (0, 1, 0, 1, 2, 3, 2, 3)
    GHALF = (0, 0, 1, 1, 0, 0, 1, 1)
    for c in range(NCORES):
        shard = state[BS * c:BS * (c + 1)]            # [64, 500]
        st = shard[:, PERM].T                         # [500, 64]
        img = np.zeros((126, 1024), np.float32)
        for u in range(8):
            rows = st[125 * DBLK[u]:125 * (DBLK[u] + 1)]
            off = 128 * u + 64 * GHALF[u]
            img[0:125, off:off + 64] = rows
        img[125, 128 * 1:128 * 1 + 64] = 1.0
        img[125, 128 * 3 + 64:128 * 3 + 128] = 1.0
        outs.append(img.astype(NP_BF16))
    return outs


def _run(L, stateTs, wa, wb, trace=False):
    nc = _get_nc(L)
    in_maps = [{"statet": np.ascontiguousarray(stateTs[c]),
                "wa": wa, "wb": wb} for c in range(NCORES)]
    res = bass_utils.run_bass_kernel_spmd(
        nc, in_maps, core_ids=list(range(NCORES)), trace=trace)
    shards = []
    sums = np.zeros((128, 2 * L), np.float64)
    for c in range(NCORES):
        hout = res.results[c]["hout"].astype(np.float32)
        shards.append(np.concatenate([hout[0:64], hout[64:128]], axis=1))
        sums += res.results[c]["sums"].astype(np.float64)
    h = np.concatenate(shards, axis=0)                # [512, 500]
    means = (sums[:, 0::2] + sums[:, 1::2]).sum(axis=0) / (B * D)  # [L]
    return h, means, res


def kernel(state, W_ih, W_hh, b_ih, b_hh, break_condition, recursion_limit):
    state = np.asarray(state, np.float32)
    L = int(np.asarray(recursion_limit))
    if L <= 0:
        return state.copy()
    bc = float(np.asarray(break_condition))

    wa, wb = _prep_weights(W_ih, W_hh, b_ih, b_hh)
    stateTs = _prep_state(state)

    h, means, _ = _run(L, stateTs, wa, wb)
    fired = np.nonzero(means > bc)[0]
    if fired.size and fired[0] + 1 < L:
        # break fired at step k* = fired[0]+1: output latches h_{k*}
        h, _, _ = _run(int(fired[0]) + 1, stateTs, wa, wb)
    return h.astype(np.float32)
